# revision 1
# baseline (speedup 1.0000x reference)
"""Trainium2 Bass kernel for the Neural-CDE-style cell (nn_JaCDE_88167088653055).

Math (per batch row b):
    x    = spline(coeffs, t)   xdot = spline(dcoeffs, t)
    l1   = x @ wx.T + h @ wh.T + b0
    relu = relu(l1);  drelu = sigmoid(l1)
    lout = relu @ wout.T + b1; th = tanh(lout); dth = 1 - th^2
    J(v) = dth * ((drelu * v) @ wout.T)        # action of the Jacobian factor
    jx   = J(xdot @ wx.T); jxh = J(jx @ wh.T); jxhh = J(jxh @ wh.T)
    out  = jx + jxh + jxhh

Device-side reformulation:
  * the [B,H,H] d_outer tensor is never materialized; every einsum with it
    collapses to per-row elementwise multiplies around small matmuls.
  * the cubic-spline evaluation folds into the wx matmul: with
    powers = dt**[0..3],  x @ wx.T == csel_flat @ (wx (x) powers).T  where
    csel_flat = coeffs[:, idx].reshape(B, CIN*4) — so the spline costs zero
    extra device passes and the contraction is K=256.
  * tanh is computed through sigmoid (tanh(x) = 2*sigmoid(2x)-1,
    1-tanh^2 = 4*s*(1-s)) so every scalar-engine activation (Relu, Sigmoid)
    lives in one ACT table set — no per-chunk activation-table reloads.
  * m1+m2+m3 accumulate in one PSUM bank via the PE (start/stop flags), so the
    final sum costs a single vector op.

Sharding: pure data parallel — batch 8192 split as 1024 rows per core across
8 cores; the small weights are replicated. All activations live
feature-major ([feature<=128 partitions, batch free]) so every matmul is
`out.T = W @ act.T` with the contraction on partitions.
"""

import numpy as np

import concourse.bass as bass
import concourse.mybir as mybir
import concourse.tile as tile
from concourse import bacc, bass_utils

N_CORES = 8
B = 8192
NOBS = 16
CIN = 64
H = 128
K4 = CIN * 4            # 256: folded (channel, power) contraction dim
BS = B // N_CORES       # 1024 batch rows per core
CHUNK = 512             # batch columns per pipeline step (one PSUM bank)
NCH = BS // CHUNK
F32 = mybir.dt.float32
F32R = mybir.dt.float32r

USE_F32R = True         # full-rate PE path; set False for exact fp32 matmuls

_NC_CACHE = {}


def _build_nc(use_f32r: bool):
    AF = mybir.ActivationFunctionType
    OP = mybir.AluOpType

    nc = bacc.Bacc("TRN2", target_bir_lowering=False, debug=False,
                   enable_asserts=False, num_devices=N_CORES)

    # dtype of everything that feeds the PE: the BIR verifier requires every
    # producer of an fp32r matmul operand to emit fp32r-typed (rounded) data.
    MMDT = F32R if use_f32r else F32

    ct = nc.dram_tensor("ct", [K4, BS], MMDT, kind="ExternalInput")
    dct = nc.dram_tensor("dct", [K4, BS], MMDT, kind="ExternalInput")
    ht = nc.dram_tensor("ht", [H, BS], MMDT, kind="ExternalInput")
    wxpt = nc.dram_tensor("wxpt", [K4, H], MMDT, kind="ExternalInput")
    wht = nc.dram_tensor("wht", [H, H], MMDT, kind="ExternalInput")
    woutt = nc.dram_tensor("woutt", [H, H], MMDT, kind="ExternalInput")
    b0c = nc.dram_tensor("b0c", [H, 1], F32, kind="ExternalInput")
    b1c2 = nc.dram_tensor("b1c2", [H, 1], F32, kind="ExternalInput")
    outt = nc.dram_tensor("outt", [H, BS], F32, kind="ExternalOutput")

    def mm(out_ap, lhsT, rhs, start=True, stop=True):
        nc.tensor.matmul(out_ap, lhsT, rhs, start=start, stop=stop,
                         skip_group_check=True)

    with tile.TileContext(nc) as tc:
        with tc.tile_pool(name="w", bufs=1) as wp, \
             tc.tile_pool(name="io", bufs=2) as io, \
             tc.tile_pool(name="tmp", bufs=2) as tmp, \
             tc.tile_pool(name="ps", bufs=1, space="PSUM") as ps:

            wxp0 = wp.tile([128, H], MMDT, tag="wxp0")
            nc.sync.dma_start(wxp0[:], wxpt[0:128, :])
            wxp1 = wp.tile([128, H], MMDT, tag="wxp1")
            nc.sync.dma_start(wxp1[:], wxpt[128:256, :])
            whs = wp.tile([H, H], MMDT, tag="whs")
            nc.sync.dma_start(whs[:], wht[:])
            wos = wp.tile([H, H], MMDT, tag="wos")
            nc.sync.dma_start(wos[:], woutt[:])
            b0s = wp.tile([H, 1], F32, tag="b0s")
            nc.sync.dma_start(b0s[:], b0c[:])
            b1s = wp.tile([H, 1], F32, tag="b1s")
            nc.sync.dma_start(b1s[:], b1c2[:])

            for ch in range(NCH):
                cs = bass.ts(ch, CHUNK)

                # spread input loads across 4 DGE queues so the first-chunk
                # loads land in ~1/4 the serialized time
                c0 = io.tile([128, CHUNK], MMDT, tag="c0")
                nc.sync.dma_start(c0[:], ct[0:128, cs])
                c1 = io.tile([128, CHUNK], MMDT, tag="c1")
                nc.scalar.dma_start(c1[:], ct[128:256, cs])
                d0 = io.tile([128, CHUNK], MMDT, tag="d0")
                nc.gpsimd.dma_start(d0[:], dct[0:128, cs])
                d1 = io.tile([128, CHUNK], MMDT, tag="d1")
                nc.sync.dma_start(d1[:], dct[128:256, cs])
                hts = io.tile([128, CHUNK], MMDT, tag="hts")
                nc.scalar.dma_start(hts[:], ht[:, cs])

                # l1.T = Wxp @ csel.T + wh @ h.T   (K = 256 + 128)
                l1 = ps.tile([H, CHUNK], F32, tag="l1")
                mm(l1[:], wxp0[:], c0[:], start=True, stop=False)
                mm(l1[:], wxp1[:], c1[:], start=False, stop=False)
                mm(l1[:], whs[:], hts[:], start=False, stop=True)

                # u.T = Wxp @ dsel.T
                u = ps.tile([H, CHUNK], F32, tag="u")
                mm(u[:], wxp0[:], d0[:], start=True, stop=False)
                mm(u[:], wxp1[:], d1[:], start=False, stop=True)

                relu = tmp.tile([H, CHUNK], MMDT, tag="relu")
                nc.scalar.activation(relu[:], l1[:], AF.Relu, bias=b0s[:, 0:1])
                drelu = tmp.tile([H, CHUNK], F32, tag="drelu")
                nc.scalar.activation(drelu[:], l1[:], AF.Sigmoid, bias=b0s[:, 0:1])

                lout = ps.tile([H, CHUNK], F32, tag="lout")
                mm(lout[:], wos[:], relu[:])

                # s = sigmoid(2*(lout + b1));  dth = 1 - tanh^2 = 4*s*(1-s) = -4*q
                # with q = s^2 - s, so  dth * x == (q * -4) * x  in one DVE op.
                s = tmp.tile([H, CHUNK], F32, tag="s")
                nc.scalar.activation(s[:], lout[:], AF.Sigmoid,
                                     bias=b1s[:, 0:1], scale=2.0)
                q = tmp.tile([H, CHUNK], F32, tag="q")
                nc.vector.scalar_tensor_tensor(q[:], s[:], 1.0, s[:],
                                               OP.subtract, OP.mult)

                p1 = tmp.tile([H, CHUNK], MMDT, tag="p1")
                nc.vector.tensor_mul(p1[:], drelu[:], u[:])
                m1 = ps.tile([H, CHUNK], F32, tag="m", bufs=3)
                mm(m1[:], wos[:], p1[:])

                jx = tmp.tile([H, CHUNK], MMDT, tag="jx")
                nc.vector.scalar_tensor_tensor(jx[:], q[:], -4.0, m1[:],
                                               OP.mult, OP.mult)
                g1 = ps.tile([H, CHUNK], F32, tag="g", bufs=2)
                mm(g1[:], whs[:], jx[:])
                p2 = tmp.tile([H, CHUNK], MMDT, tag="p2")
                nc.vector.tensor_mul(p2[:], drelu[:], g1[:])
                m2 = ps.tile([H, CHUNK], F32, tag="m", bufs=3)
                mm(m2[:], wos[:], p2[:])

                jxh = tmp.tile([H, CHUNK], MMDT, tag="jxh")
                nc.vector.scalar_tensor_tensor(jxh[:], q[:], -4.0, m2[:],
                                               OP.mult, OP.mult)
                g2 = ps.tile([H, CHUNK], F32, tag="g", bufs=2)
                mm(g2[:], whs[:], jxh[:])
                p3 = tmp.tile([H, CHUNK], MMDT, tag="p3")
                nc.vector.tensor_mul(p3[:], drelu[:], g2[:])
                m3 = ps.tile([H, CHUNK], F32, tag="m", bufs=3)
                mm(m3[:], wos[:], p3[:])

                jxhh = tmp.tile([H, CHUNK], F32, tag="jxhh")
                nc.vector.scalar_tensor_tensor(jxhh[:], q[:], -4.0, m3[:],
                                               OP.mult, OP.mult)
                # final sums on the otherwise-idle GpSimd engine (SBUF-only)
                s12 = tmp.tile([H, CHUNK], F32, tag="s12")
                nc.gpsimd.tensor_add(s12[:], jx[:], jxh[:])
                outs = tmp.tile([H, CHUNK], F32, tag="outs")
                nc.gpsimd.tensor_add(outs[:], s12[:], jxhh[:])
                nc.sync.dma_start(outt[:, cs], outs[:])

    nc.compile()
    return nc


def _get_nc():
    key = USE_F32R
    if key not in _NC_CACHE:
        _NC_CACHE[key] = _build_nc(key)
    return _NC_CACHE[key]


def _prep_in_maps(t, h, coeffs, dcoeffs, tobs, wx, wh, wout, b0, b1):
    t = np.asarray(t, np.float32)
    h = np.asarray(h, np.float32)
    coeffs = np.asarray(coeffs, np.float32)
    dcoeffs = np.asarray(dcoeffs, np.float32)
    tobs = np.asarray(tobs, np.float32)
    wx = np.asarray(wx, np.float32)
    wh = np.asarray(wh, np.float32)
    wout = np.asarray(wout, np.float32)
    b0 = np.asarray(b0, np.float32)
    b1 = np.asarray(b1, np.float32)

    ts = t[0]
    idx = int(np.clip(np.searchsorted(tobs, ts, side="right") - 1, 0, NOBS - 2))
    dtv = np.float32(ts - tobs[idx])
    powers = dtv ** np.arange(4, dtype=np.float32)            # [4]
    wxp = (wx[:, :, None] * powers[None, None, :]).reshape(H, K4)

    wxpt = np.ascontiguousarray(wxp.T)                        # [256, 128]
    wht = np.ascontiguousarray(wh.T)                          # [128, 128]
    woutt = np.ascontiguousarray(wout.T)                      # [128, 128]
    b0c = np.ascontiguousarray(b0.reshape(H, 1))
    b1c2 = np.ascontiguousarray((2.0 * b1).reshape(H, 1)).astype(np.float32)

    csel = coeffs[:, idx].reshape(B, K4)                      # [B, 256]
    dsel = dcoeffs[:, idx].reshape(B, K4)

    in_maps = []
    for c in range(N_CORES):
        sl = slice(c * BS, (c + 1) * BS)
        in_maps.append({
            "ct": np.ascontiguousarray(csel[sl].T),
            "dct": np.ascontiguousarray(dsel[sl].T),
            "ht": np.ascontiguousarray(h[sl].T),
            "wxpt": wxpt,
            "wht": wht,
            "woutt": woutt,
            "b0c": b0c,
            "b1c2": b1c2,
        })
    return in_maps


def kernel(**inputs) -> np.ndarray:
    in_maps = _prep_in_maps(**inputs)
    nc = _get_nc()
    res = bass_utils.run_bass_kernel_spmd(nc, in_maps,
                                          core_ids=list(range(N_CORES)))
    out = np.empty((B, H), np.float32)
    for c in range(N_CORES):
        out[c * BS:(c + 1) * BS] = res.results[c]["outt"].T
    return out



# revision 7
# speedup vs baseline: 1.3063x; 1.3063x over previous
"""Trainium2 Bass kernel for the Neural-CDE-style cell (nn_JaCDE_88167088653055).

Math (per batch row b):
    x    = spline(coeffs, t)   xdot = spline(dcoeffs, t)
    l1   = x @ wx.T + h @ wh.T + b0
    relu = relu(l1);  drelu = sigmoid(l1)
    lout = relu @ wout.T + b1; th = tanh(lout); dth = 1 - th^2
    J(v) = dth * ((drelu * v) @ wout.T)        # action of the Jacobian factor
    jx   = J(xdot @ wx.T); jxh = J(jx @ wh.T); jxhh = J(jxh @ wh.T)
    out  = jx + jxh + jxhh

Device-side structure (per core, batch-feature-major [H, batch] tiles):
  * spline folds into the wx matmul: x @ wx.T == csel_flat @ (wx (x) powers).T,
    so the contraction is K=256 and the spline costs no device passes.
  * dth = 1-tanh^2 computed as ACT Tanh then ACT Square (thq); the m-matmuls
    use a negated stationary -wout so every dth-multiply is a single
    scalar_tensor_tensor (thq-1)*M = dth*(wout@p).
  * m1+m2+m3 accumulate in ONE PSUM bank via PE start/stop flags; jx = dth*M1,
    t2 = dth*M2 (= jx+jxh, so jxh = t2-jx), out = dth*M3 directly.
  * the two 512-column chunks are software-pipelined stage-by-stage so every
    engine queue alternates A/B work (in-order queues never head-of-line block
    a ready chunk behind a stalled one).
  * warmup: a dummy Sigmoid first (loads the single act table covering
    Relu+Sigmoid during the input DMA) and dummy matmuls on a zeroed tile to
    ramp the PE clock out of its low p-state before real work arrives.

Sharding: pure data parallel - batch 8192 split as 1024 rows per core across
8 cores; small weights replicated; no cross-core communication.
"""

import numpy as np
import ml_dtypes

import concourse.bass as bass
import concourse.mybir as mybir
import concourse.tile as tile
from concourse import bacc, bass_utils

N_CORES = 8
B = 8192
NOBS = 16
CIN = 64
H = 128
K4 = CIN * 4            # 256: folded (channel, power) contraction dim
BS = B // N_CORES       # 1024 batch rows per core
CHUNK = 512             # batch columns per pipeline step (one PSUM bank)
NCH = BS // CHUNK       # 2
F32 = mybir.dt.float32
BF16 = mybir.dt.bfloat16
NPBF16 = ml_dtypes.bfloat16

DUMMY_MM = 4            # PE-clock warmup matmuls (N=512 each) on zeroed data

_NC_CACHE = {}


def _build_nc():
    AF = mybir.ActivationFunctionType
    OP = mybir.AluOpType

    nc = bacc.Bacc("TRN2", target_bir_lowering=False, debug=False,
                   enable_asserts=False, num_devices=N_CORES)

    # inputs packed per chunk as [c0|c1|d0|d1|h], 5*CHUNK bf16 cols per chunk
    xint = nc.dram_tensor("xint", [128, NCH * 5 * CHUNK], BF16,
                          kind="ExternalInput")
    # weights packed [wxp0|wxp1|wht|woutt|wos4t]
    wtst = nc.dram_tensor("wtst", [128, 5 * H], BF16, kind="ExternalInput")
    bst = nc.dram_tensor("bst", [H, 2], F32, kind="ExternalInput")
    outt = nc.dram_tensor("outt", [H, BS], BF16, kind="ExternalOutput")

    def mm(out_ap, lhsT, rhs, start=True, stop=True):
        nc.tensor.matmul(out_ap, lhsT, rhs, start=start, stop=stop,
                         skip_group_check=True)

    with tile.TileContext(nc) as tc:
        with tc.tile_pool(name="w", bufs=1) as wp, \
             tc.tile_pool(name="io", bufs=2) as io, \
             tc.tile_pool(name="tmp", bufs=2) as tmp, \
             tc.tile_pool(name="ps", bufs=1, space="PSUM") as ps:

            # ---- warmup + weight/bias/input loads (t=0) ----
            dmy = wp.tile([128, CHUNK], BF16, tag="dmy")
            nc.gpsimd.memset(dmy[:], 0.0)

            wts = wp.tile([128, 5 * H], BF16, tag="wts")
            nc.sync.dma_start(wts[:], wtst[:])
            bs = wp.tile([H, 2], F32, tag="bs")
            nc.sync.dma_start(bs[:], bst[:])

            xin = []
            for ch in range(NCH):
                xt = io.tile([128, 5 * CHUNK], BF16, tag="xin")
                base = ch * 5 * CHUNK
                half = 5 * CHUNK // 2  # 1280
                nc.sync.dma_start(xt[:, 0:half], xint[:, base:base + half])
                nc.scalar.dma_start(xt[:, half:5 * CHUNK],
                                    xint[:, base + half:base + 5 * CHUNK])
                xin.append(xt)

            # act-table warmup: Sigmoid first -> single table set load that
            # also covers Relu, overlapped with the input DMA
            wrm = tmp.tile([128, 1], F32, tag="wrm")
            nc.scalar.activation(wrm[:], dmy[:, 0:1], AF.Sigmoid)

            wxp0 = wts[:, 0 * H:1 * H]
            wxp1 = wts[:, 1 * H:2 * H]
            wht = wts[:, 2 * H:3 * H]
            wot = wts[:, 3 * H:4 * H]
            won = wts[:, 4 * H:5 * H]   # -wout
            b0ap = bs[:, 0:1]
            b1ap = bs[:, 1:2]

            # PE-clock warmup: dummy matmuls on zeroed data into the lo bank
            lo_warm = ps.tile([H, CHUNK], F32, tag="lo")
            for _ in range(DUMMY_MM):
                mm(lo_warm[:], dmy[:, 0:H], dmy[:])

            # ---- software-pipelined chunk stages ----
            l1 = [None] * NCH
            u = [None] * NCH
            relu = [None] * NCH
            drelu = [None] * NCH
            lo = [None] * NCH
            q = [None] * NCH
            m = [None] * NCH
            g = [None] * NCH
            jx = [None] * NCH

            # F: front-end matmuls + relu/drelu
            for ch in range(NCH):
                xt = xin[ch]
                c0 = xt[:, 0 * CHUNK:1 * CHUNK]
                c1 = xt[:, 1 * CHUNK:2 * CHUNK]
                d0 = xt[:, 2 * CHUNK:3 * CHUNK]
                d1 = xt[:, 3 * CHUNK:4 * CHUNK]
                ht = xt[:, 4 * CHUNK:5 * CHUNK]
                l1[ch] = ps.tile([H, CHUNK], F32, tag="l1", bufs=2, name="l1")
                u[ch] = ps.tile([H, CHUNK], F32, tag="u", bufs=2, name="u")
                mm(l1[ch][:], wxp0, c0, start=True, stop=False)
                mm(u[ch][:], wxp0, d0, start=True, stop=False)
                mm(l1[ch][:], wxp1, c1, start=False, stop=False)
                mm(u[ch][:], wxp1, d1, start=False, stop=True)
                mm(l1[ch][:], wht, ht, start=False, stop=True)
                relu[ch] = tmp.tile([H, CHUNK], BF16, tag="relu", name="relu")
                nc.scalar.activation(relu[ch][:], l1[ch][:], AF.Relu,
                                     bias=b0ap)
                drelu[ch] = tmp.tile([H, CHUNK], F32, tag="drelu", name="drelu")
                nc.scalar.activation(drelu[ch][:], l1[ch][:], AF.Sigmoid,
                                     bias=b0ap)

            # C1: lout, s, q, p1, m1
            for ch in range(NCH):
                lo[ch] = ps.tile([H, CHUNK], F32, tag="lo", name="lo")
                mm(lo[ch][:], wot, relu[ch][:])
                th = tmp.tile([H, CHUNK], F32, tag="th")
                nc.scalar.activation(th[:], lo[ch][:], AF.Tanh, bias=b1ap)
                q[ch] = tmp.tile([H, CHUNK], F32, tag="q", name="q")
                nc.scalar.activation(q[ch][:], th[:], AF.Square)
                p1 = tmp.tile([H, CHUNK], BF16, tag="p1")
                nc.vector.tensor_mul(p1[:], drelu[ch][:], u[ch][:])
                m[ch] = ps.tile([H, CHUNK], F32, tag="m", bufs=2, name="m")
                mm(m[ch][:], won, p1[:], start=True, stop=False)

            # C2: jx = dth*M1, g1, p2, m2  (dth = -4q; 4 folded into wo4)
            for ch in range(NCH):
                jx[ch] = tmp.tile([H, CHUNK], BF16, tag="jx", name="jx")
                nc.vector.scalar_tensor_tensor(jx[ch][:], q[ch][:], 1.0,
                                               m[ch][:], OP.subtract, OP.mult)
                g[ch] = ps.tile([H, CHUNK], F32, tag="g", name="g")
                mm(g[ch][:], wht, jx[ch][:])
                p2 = tmp.tile([H, CHUNK], BF16, tag="p2")
                nc.vector.tensor_mul(p2[:], drelu[ch][:], g[ch][:])
                mm(m[ch][:], won, p2[:], start=False, stop=False)

            # C3: t2 = dth*M2 = jx+jxh, jxh = t2-jx, g2, p3, m3
            for ch in range(NCH):
                t2 = tmp.tile([H, CHUNK], BF16, tag="t2")
                nc.vector.scalar_tensor_tensor(t2[:], q[ch][:], 1.0,
                                               m[ch][:], OP.subtract, OP.mult)
                jxh = tmp.tile([H, CHUNK], BF16, tag="jxh")
                nc.gpsimd.tensor_sub(jxh[:], t2[:], jx[ch][:])
                g2 = ps.tile([H, CHUNK], F32, tag="g")
                mm(g2[:], wht, jxh[:])
                p3 = tmp.tile([H, CHUNK], BF16, tag="p3")
                nc.vector.tensor_mul(p3[:], drelu[ch][:], g2[:])
                mm(m[ch][:], won, p3[:], start=False, stop=True)

            # OUT: out = dth*M3 = jx+jxh+jxhh
            for ch in range(NCH):
                outs = tmp.tile([H, CHUNK], BF16, tag="outs")
                nc.vector.scalar_tensor_tensor(outs[:], q[ch][:], 1.0,
                                               m[ch][:], OP.subtract, OP.mult)
                cs = bass.ts(ch, CHUNK)
                nc.sync.dma_start(outt[:, cs], outs[:])

    nc.compile()
    return nc


def _get_nc():
    if "nc" not in _NC_CACHE:
        _NC_CACHE["nc"] = _build_nc()
    return _NC_CACHE["nc"]


def _prep_in_maps(t, h, coeffs, dcoeffs, tobs, wx, wh, wout, b0, b1):
    t = np.asarray(t, np.float32)
    h = np.asarray(h, np.float32)
    coeffs = np.asarray(coeffs, np.float32)
    dcoeffs = np.asarray(dcoeffs, np.float32)
    tobs = np.asarray(tobs, np.float32)
    wx = np.asarray(wx, np.float32)
    wh = np.asarray(wh, np.float32)
    wout = np.asarray(wout, np.float32)
    b0 = np.asarray(b0, np.float32)
    b1 = np.asarray(b1, np.float32)

    ts = t[0]
    idx = int(np.clip(np.searchsorted(tobs, ts, side="right") - 1, 0, NOBS - 2))
    dtv = np.float32(ts - tobs[idx])
    powers = dtv ** np.arange(4, dtype=np.float32)            # [4]
    wxp = (wx[:, :, None] * powers[None, None, :]).reshape(H, K4)

    wxpt = wxp.T                                              # [256, 128]
    # packed stationaries: [wxp0|wxp1|wht|woutt|4*woutt], bf16
    wts = np.concatenate(
        [wxpt[0:128], wxpt[128:256], wh.T, wout.T, -wout.T],
        axis=1).astype(NPBF16)
    bst = np.stack([b0, b1], axis=1).astype(np.float32)        # [H, 2]

    csel = coeffs[:, idx].reshape(B, K4)                      # [B, 256]
    dsel = dcoeffs[:, idx].reshape(B, K4)
    cselT = csel.T.astype(NPBF16)                             # [256, B]
    dselT = dsel.T.astype(NPBF16)
    hT = h.T.astype(NPBF16)                                   # [128, B]

    in_maps = []
    for c in range(N_CORES):
        sl = slice(c * BS, (c + 1) * BS)
        blocks = []
        for ch in range(NCH):
            s2 = slice(c * BS + ch * CHUNK, c * BS + (ch + 1) * CHUNK)
            blocks += [cselT[0:128, s2], cselT[128:256, s2],
                       dselT[0:128, s2], dselT[128:256, s2], hT[:, s2]]
        xint = np.ascontiguousarray(np.concatenate(blocks, axis=1))
        in_maps.append({"xint": xint, "wtst": wts, "bst": bst})
    return in_maps


def kernel(**inputs) -> np.ndarray:
    in_maps = _prep_in_maps(**inputs)
    nc = _get_nc()
    res = bass_utils.run_bass_kernel_spmd(nc, in_maps,
                                          core_ids=list(range(N_CORES)))
    out = np.empty((B, H), np.float32)
    for c in range(N_CORES):
        out[c * BS:(c + 1) * BS] = res.results[c]["outt"].astype(np.float32).T
    return out
